# revision 2
# baseline (speedup 1.0000x reference)
"""Trainium2 Bass kernel for the MAB problem — v3.

v3 changes vs v2:
  - FFN fused over raw Hm: F = W1^T Hm + ws (x) negmean, then column-scale
    by rstd — FFN matmuls start immediately after phase A, before the LN
    chain finishes. ws = colsum(W1) precomputed on host.
  - Shorter LN chains: nm_bc = -s1/D via ACT copy (frees s1 psum), var via
    two DVE scalar_tensor_tensor ops, rstd via ACT Rsqrt directly.
  - yv host-preswizzled to the exact SBUF layout (contiguous per-partition
    DMA descriptors, bf16).
  - First xt/yt chunk split so the first S matmul starts earlier.
  - Output DMA split across the SP and ACT hardware DGE queues.
"""

import functools
import math
import sys

import numpy as np

sys.path.insert(0, "/opt/trn_rl_repo")

import concourse.bass as bass  # noqa: E402
import concourse.tile as tile  # noqa: E402
from concourse import bacc, mybir  # noqa: E402
from concourse.bass_utils import run_bass_kernel_spmd  # noqa: E402

F32 = mybir.dt.float32
F32R = mybir.dt.float32r
BF16 = mybir.dt.bfloat16
AF = mybir.ActivationFunctionType
OP = mybir.AluOpType

P = 128
DIM = 1024
NT = 1024
H = 16
D = 64
NC = DIM // P
NMC = NT // P
EPS = 1e-5
SCALE = 1.0 / math.sqrt(DIM)


def _pbcast2(ap, reps):
    """[2, N] AP -> [2*reps, N]: each row partition-broadcast reps times."""
    part = ap.ap[0]
    return bass.AP(
        tensor=ap.tensor,
        offset=ap.offset,
        ap=[list(part), [0, reps]] + [list(d) for d in ap.ap[1:]],
    )


def build_program(n_cores: int, reps: int = 1):
    nc = bacc.Bacc(
        "TRN2",
        target_bir_lowering=False,
        debug=False,
        num_devices=n_cores,
    )

    xt_d = nc.dram_tensor("xt", [DIM, NT], F32R, kind="ExternalInput").ap()
    yt_d = nc.dram_tensor("yt", [DIM, NT], F32R, kind="ExternalInput").ap()
    yv_d = nc.dram_tensor(
        "yv", [P, H, NMC, D + 1], BF16, kind="ExternalInput"
    ).ap()
    w1_d = nc.dram_tensor("w1", [DIM, DIM], F32R, kind="ExternalInput").ap()
    b1_d = nc.dram_tensor("b1", [DIM], F32, kind="ExternalInput").ap()
    ws_d = nc.dram_tensor("ws", [1, DIM], F32R, kind="ExternalInput").ap()
    ot_d = nc.dram_tensor("ot", [DIM, NT], F32R, kind="ExternalOutput").ap()
    st1_d = nc.dram_tensor("st1", [1, NT], F32, kind="ExternalOutput").ap()
    st2_d = nc.dram_tensor("st2", [1, NT], F32, kind="ExternalOutput").ap()
    rc_dram = nc.dram_tensor("rc_dram", [NC, 2, NT], F32).ap()

    xt_r = xt_d.rearrange("(c p) n -> p c n", p=P)
    yt_r = yt_d.rearrange("(c p) n -> p c n", p=P)
    w1_r = w1_d.rearrange("(kc p) o -> p kc o", p=P)
    b1_r = b1_d.rearrange("(c p) -> p c", p=P)
    ot_r = ot_d.rearrange("(c p) n -> p c n", p=P)

    with tile.TileContext(nc) as tc:
        _frees = []

        def _single(shape, name, dtype=F32):
            t, free = tc.tile(shape, dtype, name=name)
            _frees.append(free)
            return t

        xt_sb = _single([P, NC, NT], "xt_sb", F32R)   # X^T, later Hn
        yt_sb = _single([P, NC, NT], "yt_sb", F32R)   # Y^T, later W1
        yv_sb = _single([P, H, NMC, D + 1], "yv_sb", BF16)
        ht_sb = _single([P, NC, NT], "ht_sb", F32R)   # Hm, later O
        sq_sb = _single([P, NC, NT], "sq_sb", BF16)   # squares for LN stats
        w1_sb = yt_sb
        b1_sb = _single([P, NC], "b1_sb")
        ws_sb = _single([1, DIM], "ws_sb", F32R)
        ones_sb = _single([P, P], "ones_sb", F32R)
        ones_bf = _single([P, P], "ones_bf", BF16)
        eps_t = _single([P, 1], "eps_t")

        warm_t = _single([1, 1], "warm_t")
        nc.vector.memset(eps_t, EPS)
        nc.vector.memset(ones_sb.bitcast(F32), 1.0)
        nc.vector.memset(ones_bf, 1.0)
        # load the Exp ACT table while the first input DMAs run
        nc.scalar.activation(warm_t, eps_t[0:1, :], AF.Exp)

        import contextlib
        loop_cm = tc.For_i(0, reps, 1) if reps > 1 else contextlib.nullcontext()
        with loop_cm:
            # ---- input DMAs, priority order (sync + gpsimd queues) ----
            nc.sync.dma_start(out=yt_sb[:, 0, 0:128], in_=yt_r[:, 0, 0:128])
            nc.sync.dma_start(out=xt_sb[:, 0, 0:512], in_=xt_r[:, 0, 0:512])
            nc.sync.dma_start(out=xt_sb[:, 0, 512:], in_=xt_r[:, 0, 512:])
            nc.gpsimd.dma_start(out=yv_sb[:, 0:2, :, :], in_=yv_d[:, 0:2, :, :])
            nc.sync.dma_start(out=yt_sb[:, 0, 128:], in_=yt_r[:, 0, 128:])
            nc.sync.dma_start(out=xt_sb[:, 1:2, :], in_=xt_r[:, 1:2, :])
            nc.sync.dma_start(out=yt_sb[:, 1:2, :], in_=yt_r[:, 1:2, :])
            nc.gpsimd.dma_start(out=yv_sb[:, 2:4, :, :], in_=yv_d[:, 2:4, :, :])
            nc.sync.dma_start(out=xt_sb[:, 2:4, :], in_=xt_r[:, 2:4, :])
            nc.sync.dma_start(out=yt_sb[:, 2:4, :], in_=yt_r[:, 2:4, :])
            nc.gpsimd.dma_start(out=yv_sb[:, 4:8, :, :], in_=yv_d[:, 4:8, :, :])
            nc.sync.dma_start(out=xt_sb[:, 4:8, :], in_=xt_r[:, 4:8, :])
            nc.sync.dma_start(out=yt_sb[:, 4:8, :], in_=yt_r[:, 4:8, :])
            nc.gpsimd.dma_start(out=yv_sb[:, 8:16, :, :], in_=yv_d[:, 8:16, :, :])
            nc.gpsimd.dma_start(out=b1_sb, in_=b1_r)
            nc.gpsimd.dma_start(out=ws_sb, in_=ws_d)

            with (
                tc.tile_pool(name="psum_s", bufs=2, space="PSUM") as ps_pool,
                tc.tile_pool(name="psum_a", bufs=2, space="PSUM") as pa_pool,
                tc.tile_pool(name="work", bufs=3) as work,
                tc.tile_pool(name="pairs", bufs=2) as pairs,
                tc.tile_pool(name="vec", bufs=4) as vec,
                tc.tile_pool(name="stat", bufs=4) as stat,
            ):
                # =============== Phase A: attention ===============
                pend = None
                attn_ps = {}
                av2 = {}
                rc2 = {}
                pend_pair = []

                def pair_epilogue(ct, fast=False):
                    if fast:
                        # st slots free at phase A tail: K=1 PE broadcasts
                        # (one per head, dst partitions 0:D) avoid the DRAM
                        # roundtrip latency; the muls below are
                        # partition-shifted reads (v1-proven legal on DVE)
                        rbs = []
                        for hh in range(2):
                            rbh = ps_pool.tile(
                                [P, NT], F32, tag="st", name=f"rbps{hh}"
                            )
                            for nh in range(2):
                                sl = slice(nh * 512, (nh + 1) * 512)
                                nc.tensor.matmul(
                                    rbh[0:D, sl],
                                    ones_sb[0:1, 0:D],
                                    rc2[(ct, hh)][:, sl],
                                    start=True,
                                    stop=True,
                                )
                            rbs.append(rbh)
                    else:
                        nc.gpsimd.dma_start(
                            out=rc_dram[ct, 0:1, :], in_=rc2[(ct, 0)].bitcast(F32)
                        )
                        nc.gpsimd.dma_start(
                            out=rc_dram[ct, 1:2, :], in_=rc2[(ct, 1)].bitcast(F32)
                        )
                        rb = pairs.tile([P, NT], F32, tag="rb", name=f"rb{ct}")
                        nc.gpsimd.dma_start(
                            out=rb, in_=_pbcast2(rc_dram[ct, :, :], D)
                        )
                    dst = ht_sb[:, ct, :]
                    dstf = ht_sb[:, ct, :].bitcast(F32)
                    if fast:
                        nc.vector.tensor_mul(
                            ht_sb[0:D, ct, :], av2[ct][0:D, :], rbs[0][0:D, :]
                        )
                        nc.vector.tensor_mul(
                            ht_sb[D:P, ct, :], av2[ct][D:P, :], rbs[1][0:D, :]
                        )
                    else:
                        nc.vector.tensor_mul(dst, av2[ct], rb)
                    nc.gpsimd.tensor_add(
                        dst, dstf, xt_sb[:, ct, :].bitcast(F32)
                    )
                    nc.vector.tensor_mul(sq_sb[:, ct, :], dstf, dstf)
                    # yt chunk ct is dead now -> stream in the W1 chunk
                    nc.sync.dma_start(
                        out=w1_sb[:, ct, :], in_=w1_r[:, ct, :]
                    )

                for k in range(H * NMC + 1):
                    if k < H * NMC:
                        h, mc = divmod(k, NMC)
                        ct, off = h // 2, (h % 2) * D
                        st = ps_pool.tile([P, NT], F32, tag="st")
                        lhsT = yt_sb[off : off + D, ct, mc * P : (mc + 1) * P]
                        for nh in range(2):
                            sl = slice(nh * 512, (nh + 1) * 512)
                            nc.tensor.matmul(
                                st[:, sl],
                                lhsT,
                                xt_sb[off : off + D, ct, sl],
                                start=True,
                                stop=True,
                            )
                        e = work.tile([P, NT], BF16, tag="e")
                        nc.scalar.activation(e, st, AF.Exp, scale=SCALE)
                        cur = (h, mc, e)
                    else:
                        cur = None

                    if pend is not None:
                        h, mc, e = pend
                        ct, lo = h // 2, (h % 2) * D
                        if mc == 0:
                            attn_ps[h] = pa_pool.tile(
                                [D + 1, NT], F32, tag="at", name=f"at{h}"
                            )
                        ap_t = attn_ps[h]
                        lv = yv_sb[:, h, mc, :]
                        for nh in range(2):
                            sl = slice(nh * 512, (nh + 1) * 512)
                            nc.tensor.matmul(
                                ap_t[:, sl],
                                lv,
                                e[:, sl],
                                start=(mc == 0),
                                stop=(mc == NMC - 1),
                            )
                        if mc == NMC - 1:
                            if h % 2 == 0:
                                av2[ct] = pairs.tile(
                                    [P, NT], F32, tag="av", name=f"av2_{ct}"
                                )
                            rc2[(ct, h % 2)] = vec.tile(
                                [1, NT], F32R, tag="rc", name=f"rc_{h}"
                            )
                            nc.vector.tensor_copy(
                                av2[ct][lo : lo + D, :], ap_t[0:D, :]
                            )
                            with nc.allow_low_precision(reason="f32r denom"):
                                nc.vector.reciprocal(
                                    rc2[(ct, h % 2)], ap_t[D : D + 1, :]
                                )
                            del attn_ps[h]
                            if h % 2 == 1:
                                pend_pair.append((ct, k))
                    while pend_pair and (
                        cur is None or k - pend_pair[0][1] >= 4
                    ):
                        ctp, _ = pend_pair.pop(0)
                        pair_epilogue(ctp, fast=(ctp == NC - 1))
                    pend = cur

                # ====== LN stats (broadcast form) + short chain ======
                def ln_stats(src_sb, sqr_sb, idx, interleave=None):
                    """s1/s2 accumulation matmuls. If interleave is None runs
                    all chunks now; else caller drives per-chunk via the
                    returned closure. Returns (s1, s2) psum tiles."""
                    s1 = pa_pool.tile([P, NT], F32, tag="at", name=f"s1_{idx}")
                    s2 = pa_pool.tile([P, NT], F32, tag="at", name=f"s2_{idx}")

                    def chunk(ct, which=None):
                        if which in (None, 0):
                            for nh in range(2):
                                sl = slice(nh * 512, (nh + 1) * 512)
                                nc.tensor.matmul(
                                    s1[:, sl], ones_sb, src_sb[:, ct, sl],
                                    start=(ct == 0), stop=(ct == NC - 1),
                                )
                        if which in (None, 1):
                            for nh in range(2):
                                sl = slice(nh * 512, (nh + 1) * 512)
                                nc.tensor.matmul(
                                    s2[:, sl], ones_bf, sqr_sb[:, ct, sl],
                                    start=(ct == 0), stop=(ct == NC - 1),
                                )

                    if interleave is None:
                        for ct in range(NC):
                            chunk(ct)
                    return s1, s2, chunk

                def ln_chain(s1, s2, idx):
                    """Returns (nm_bc, rs_bc): [P,NT] broadcast -mean and
                    rstd. Frees s1/s2 psums early."""
                    c = 1.0 / DIM
                    nm_bc = stat.tile([P, NT], F32R, tag="st", name=f"nm_{idx}")
                    v_bc = stat.tile([P, NT], F32, tag="st", name=f"v_{idx}")
                    rs_bc = stat.tile([P, NT], F32R, tag="st", name=f"r_{idx}")
                    with nc.allow_low_precision(reason="f32r -mean"):
                        nc.scalar.activation(nm_bc, s1, AF.Copy, scale=-c)
                    nmf = nm_bc.bitcast(F32)
                    nc.vector.scalar_tensor_tensor(
                        v_bc, nmf, -1.0, nmf, OP.mult, OP.mult
                    )
                    nc.vector.scalar_tensor_tensor(
                        v_bc, s2, c, v_bc, OP.mult, OP.add
                    )
                    nc.scalar.activation(v_bc, v_bc, AF.Sqrt, bias=eps_t)
                    with nc.allow_low_precision(reason="f32r rstd"):
                        nc.vector.reciprocal(rs_bc, v_bc)
                    return nm_bc, rs_bc

                s1h, s2h, h_chunk = ln_stats(ht_sb, sq_sb, 0, interleave=True)
                for ct in range(NC - 1):
                    h_chunk(ct, which=0)
                for ct in range(NC - 1):
                    h_chunk(ct, which=1)

                # ====== Phase C: fused FFN on raw Hm + LN_o stats ======
                # O goes into xt_sb (over Hn; each Hn chunk's only reader is
                # its own residual add) so FFN's Hm rhs is never clobbered.
                # PE order: F(0), h_stats(7), F(1), rank1(0), F(2), rank1(1),
                # o_stats(0), ... so PE never waits on the DVE/ACT pipeline.
                s1o, s2o, o_chunk = ln_stats(xt_sb, sq_sb, 1, interleave=True)
                nm_h = rs_h = nm_row = None

                def hn(ct):
                    # chunks 0/1 go to Pool so the DVE queue stays clear for
                    # u(0) (which gates FFN psum recycling)
                    dst = xt_sb[:, ct, :]
                    dstf = xt_sb[:, ct, :].bitcast(F32)
                    eng = nc.gpsimd if ct in (0, 1, 4, 6) else nc.vector
                    eng.tensor_add(
                        dst, ht_sb[:, ct, :].bitcast(F32), nm_h.bitcast(F32)
                    )
                    eng.tensor_mul(dst, dstf, rs_h.bitcast(F32))

                def rank1(oc):
                    for nh in range(2):
                        sl = slice(nh * 512, (nh + 1) * 512)
                        nc.tensor.matmul(
                            fps[oc][:, sl],
                            ws_sb[:, oc * P : (oc + 1) * P],
                            nm_row[:, sl],
                            start=False,
                            stop=True,
                        )

                def fcp(oc):
                    # ACT evacuates the FFN psum so its slot recycles at ACT
                    # pace instead of waiting on the DVE chain
                    t = work.tile([P, NT], F32, tag="f", name=f"fc{oc}")
                    nc.scalar.activation(t, fps[oc], AF.Copy)
                    del fps[oc]
                    fcps[oc] = t

                def finish_u(oc):
                    u = work.tile([P, NT], F32, tag="e", name=f"u{oc}")
                    nc.vector.tensor_mul(u, fcps.pop(oc), rs_h.bitcast(F32))
                    r = work.tile([P, NT], F32, tag="e", name=f"r{oc}")
                    nc.scalar.activation(
                        r, u, AF.Relu, bias=b1_sb[:, oc : oc + 1]
                    )
                    return r

                def finish_res(oc, r):
                    dst = xt_sb[:, oc, :]
                    dstf = xt_sb[:, oc, :].bitcast(F32)
                    eng = nc.gpsimd if oc in (1, 4, 6) else nc.vector
                    eng.tensor_add(dst, dstf, r)
                    eng2 = nc.vector if oc in (1, 4, 6) else nc.gpsimd
                    eng2.tensor_mul(sq_sb[:, oc, :], dstf, dstf)
                    oq = nc.sync if oc % 2 == 0 else nc.scalar
                    oq.dma_start(out=ot_r[:, oc, :], in_=xt_sb[:, oc, :])

                fps = {}
                fcps = {}
                for oc in range(NC):
                    fps[oc] = ps_pool.tile([P, NT], F32, tag="st", name=f"fps{oc}")
                    for kc in range(NC):
                        lhsT = w1_sb[:, kc, oc * P : (oc + 1) * P]
                        for nh in range(2):
                            sl = slice(nh * 512, (nh + 1) * 512)
                            nc.tensor.matmul(
                                fps[oc][:, sl],
                                lhsT,
                                ht_sb[:, kc, sl],
                                start=(kc == 0),
                                stop=False,
                            )
                    if oc == 0:
                        c = 1.0 / DIM
                        h_chunk(NC - 1, which=0)
                        # chain part A: only needs the s1 stop
                        nm_h = stat.tile([P, NT], F32R, tag="st", name="nm_h")
                        v_bc = stat.tile([P, NT], F32, tag="st", name="v_h")
                        rs_h = stat.tile([P, NT], F32R, tag="st", name="rs_h")
                        with nc.allow_low_precision(reason="f32r -mean"):
                            nc.scalar.activation(nm_h, s1h, AF.Copy, scale=-c)
                        nmf = nm_h.bitcast(F32)
                        nc.vector.scalar_tensor_tensor(
                            v_bc, nmf, -1.0, nmf, OP.mult, OP.mult
                        )
                        nm_row = nm_h[0:1, :]
                        rank1(0)
                        fcp(0)
                        h_chunk(NC - 1, which=1)
                        # chain part B: needs the s2 stop
                        nc.vector.scalar_tensor_tensor(
                            v_bc, s2h, c, v_bc, OP.mult, OP.add
                        )
                        nc.scalar.activation(v_bc, v_bc, AF.Sqrt, bias=eps_t)
                        with nc.allow_low_precision(reason="f32r rstd"):
                            nc.vector.reciprocal(rs_h, v_bc)
                        hn(0)
                    else:
                        rank1(oc)
                        fcp(oc)
                        rr = finish_u(oc - 1)
                        hn(oc)
                        finish_res(oc - 1, rr)
                        if oc >= 2:
                            o_chunk(oc - 2)
                rr = finish_u(NC - 1)
                finish_res(NC - 1, rr)
                o_chunk(NC - 2)
                o_chunk(NC - 1)

                # ====== Phase D: ship raw stat rows; host applies LN_o ====
                r1 = vec.tile([1, NT], F32, tag="rc", name="r1")
                r2 = vec.tile([1, NT], F32, tag="rc", name="r2")
                nc.vector.tensor_copy(r1, s1o[0:1, :])
                nc.vector.tensor_copy(r2, s2o[0:1, :])
                nc.sync.dma_start(out=st1_d, in_=r1)
                nc.scalar.dma_start(out=st2_d, in_=r2)

        for free in reversed(_frees):
            free()

    nc.finalize()
    return nc


@functools.lru_cache(maxsize=4)
def _program(n_cores: int, reps: int = 1):
    return build_program(n_cores, reps)


def _prep_core(Xb, Yb):
    import ml_dtypes

    xt = np.ascontiguousarray(Xb.T)
    yt = np.ascontiguousarray(Yb.T)
    # [P, H, NMC, D+1]: partition-major, matching the SBUF tile exactly
    yv = np.empty((P, H, NMC, D + 1), np.float32)
    v = Yb.reshape(NMC, P, H, D)  # m = mc*128 + p
    yv[:, :, :, :D] = v.transpose(1, 2, 0, 3)
    yv[:, :, :, D] = 1.0
    return xt, yt, yv.astype(ml_dtypes.bfloat16)


def kernel(X, Y, W1, b1, gamma_h, beta_h, gamma_o, beta_o, num_heads):
    X = np.asarray(X, np.float32)
    Y = np.asarray(Y, np.float32)
    W1 = np.asarray(W1, np.float32)
    b1 = np.asarray(b1, np.float32)
    gamma_h = np.asarray(gamma_h, np.float32)
    beta_h = np.asarray(beta_h, np.float32)
    gamma_o = np.asarray(gamma_o, np.float32)
    beta_o = np.asarray(beta_o, np.float32)
    B, n, dim = X.shape
    assert (B, n, dim) == (8, NT, DIM) and int(num_heads) == H

    affine_h = bool(not (np.all(gamma_h == 1.0) and np.all(beta_h == 0.0)))
    affine_o = bool(not (np.all(gamma_o == 1.0) and np.all(beta_o == 0.0)))
    assert not affine_h, "v3 kernel only supports non-affine LN_h"

    ws = W1.sum(axis=0, keepdims=True)
    nc = _program(B)
    in_maps = []
    for b in range(B):
        xt, yt, yv = _prep_core(X[b], Y[b])
        in_maps.append(
            {"xt": xt, "yt": yt, "yv": yv, "w1": W1, "b1": b1, "ws": ws}
        )

    res = run_bass_kernel_spmd(nc, in_maps, list(range(B)))

    out = np.empty((B, NT, DIM), np.float32)
    for b in range(B):
        s1 = res.results[b]["st1"][0].astype(np.float64)
        s2 = res.results[b]["st2"][0].astype(np.float64)
        mean = s1 / DIM
        var = s2 / DIM - mean * mean
        rs = (1.0 / np.sqrt(var + EPS)).astype(np.float32)[:, None]
        bv = (-mean).astype(np.float32)[:, None] * rs
        o = res.results[b]["ot"].T * rs + bv
        if affine_o:
            o = o * gamma_o[None, :] + beta_o[None, :]
        out[b] = o
    return out


# revision 7
# speedup vs baseline: 1.0676x; 1.0676x over previous
"""Trainium2 Bass kernel for the MAB problem — v3.

v3 changes vs v2:
  - FFN fused over raw Hm: F = W1^T Hm + ws (x) negmean, then column-scale
    by rstd — FFN matmuls start immediately after phase A, before the LN
    chain finishes. ws = colsum(W1) precomputed on host.
  - Shorter LN chains: nm_bc = -s1/D via ACT copy (frees s1 psum), var via
    two DVE scalar_tensor_tensor ops, rstd via ACT Rsqrt directly.
  - yv host-preswizzled to the exact SBUF layout (contiguous per-partition
    DMA descriptors, bf16).
  - First xt/yt chunk split so the first S matmul starts earlier.
  - Output DMA split across the SP and ACT hardware DGE queues.
"""

import functools
import math
import sys

import numpy as np

sys.path.insert(0, "/opt/trn_rl_repo")

import concourse.bass as bass  # noqa: E402
import concourse.tile as tile  # noqa: E402
from concourse import bacc, mybir  # noqa: E402
from concourse.bass_utils import run_bass_kernel_spmd  # noqa: E402

F32 = mybir.dt.float32
F32R = mybir.dt.float32r
BF16 = mybir.dt.bfloat16
AF = mybir.ActivationFunctionType
OP = mybir.AluOpType

P = 128
DIM = 1024
NT = 1024
H = 16
D = 64
NC = DIM // P
NMC = NT // P
EPS = 1e-5
SCALE = 1.0 / math.sqrt(DIM)


def _pbcast2(ap, reps):
    """[2, N] AP -> [2*reps, N]: each row partition-broadcast reps times."""
    part = ap.ap[0]
    return bass.AP(
        tensor=ap.tensor,
        offset=ap.offset,
        ap=[list(part), [0, reps]] + [list(d) for d in ap.ap[1:]],
    )


def build_program(n_cores: int, reps: int = 1):
    nc = bacc.Bacc(
        "TRN2",
        target_bir_lowering=False,
        debug=False,
        num_devices=n_cores,
    )

    xt_d = nc.dram_tensor("xt", [DIM, NT], F32R, kind="ExternalInput").ap()
    yt_d = nc.dram_tensor("yt", [DIM, NT], F32R, kind="ExternalInput").ap()
    yv_d = nc.dram_tensor(
        "yv", [P, H, NMC, D + 1], BF16, kind="ExternalInput"
    ).ap()
    w1_d = nc.dram_tensor("w1", [DIM, DIM], F32R, kind="ExternalInput").ap()
    b1_d = nc.dram_tensor("b1", [DIM], F32, kind="ExternalInput").ap()
    ws_d = nc.dram_tensor("ws", [1, DIM], F32R, kind="ExternalInput").ap()
    ot_d = nc.dram_tensor("ot", [DIM, NT], F32R, kind="ExternalOutput").ap()
    st1_d = nc.dram_tensor("st1", [1, NT], F32, kind="ExternalOutput").ap()
    st2_d = nc.dram_tensor("st2", [1, NT], F32, kind="ExternalOutput").ap()
    rc_dram = nc.dram_tensor("rc_dram", [NC, 2, NT], F32).ap()

    xt_r = xt_d.rearrange("(c p) n -> p c n", p=P)
    yt_r = yt_d.rearrange("(c p) n -> p c n", p=P)
    w1_r = w1_d.rearrange("(kc p) o -> p kc o", p=P)
    b1_r = b1_d.rearrange("(c p) -> p c", p=P)
    ot_r = ot_d.rearrange("(c p) n -> p c n", p=P)

    with tile.TileContext(nc) as tc:
        _frees = []

        def _single(shape, name, dtype=F32):
            t, free = tc.tile(shape, dtype, name=name)
            _frees.append(free)
            return t

        xt_sb = _single([P, NC, NT], "xt_sb", F32R)   # X^T, later Hn
        yt_sb = _single([P, NC, NT], "yt_sb", F32R)   # Y^T, later W1
        yv_sb = _single([P, H, NMC, D + 1], "yv_sb", BF16)
        ht_sb = _single([P, NC, NT], "ht_sb", F32R)   # Hm, later O
        sq_sb = _single([P, NC, NT], "sq_sb", BF16)   # squares for LN stats
        w1_sb = yt_sb
        b1_sb = _single([P, NC], "b1_sb")
        ws_sb = _single([1, DIM], "ws_sb", F32R)
        ones_sb = _single([P, P], "ones_sb", F32R)
        ones_bf = _single([P, P], "ones_bf", BF16)
        eps_t = _single([P, 1], "eps_t")

        warm_t = _single([1, 1], "warm_t")
        nc.vector.memset(eps_t, EPS)
        nc.vector.memset(ones_sb.bitcast(F32), 1.0)
        nc.vector.memset(ones_bf, 1.0)
        # load the Exp ACT table while the first input DMAs run
        nc.scalar.activation(warm_t, eps_t[0:1, :], AF.Exp)

        import contextlib
        loop_cm = tc.For_i(0, reps, 1) if reps > 1 else contextlib.nullcontext()
        with loop_cm:
            # ---- input DMAs, priority order (sync + gpsimd queues) ----
            nc.sync.dma_start(out=yt_sb[:, 0, 0:128], in_=yt_r[:, 0, 0:128])
            nc.scalar.dma_start(out=xt_sb[:, 0, 0:512], in_=xt_r[:, 0, 0:512])
            nc.sync.dma_start(out=xt_sb[:, 0, 512:], in_=xt_r[:, 0, 512:])
            nc.gpsimd.dma_start(out=yv_sb[:, 0:2, :, :], in_=yv_d[:, 0:2, :, :])
            nc.sync.dma_start(out=yt_sb[:, 0, 128:], in_=yt_r[:, 0, 128:])
            nc.sync.dma_start(out=xt_sb[:, 1:2, :], in_=xt_r[:, 1:2, :])
            nc.sync.dma_start(out=yt_sb[:, 1:2, :], in_=yt_r[:, 1:2, :])
            nc.gpsimd.dma_start(out=yv_sb[:, 2:4, :, :], in_=yv_d[:, 2:4, :, :])
            nc.sync.dma_start(out=xt_sb[:, 2:4, :], in_=xt_r[:, 2:4, :])
            nc.sync.dma_start(out=yt_sb[:, 2:4, :], in_=yt_r[:, 2:4, :])
            nc.gpsimd.dma_start(out=yv_sb[:, 4:8, :, :], in_=yv_d[:, 4:8, :, :])
            nc.sync.dma_start(out=xt_sb[:, 4:8, :], in_=xt_r[:, 4:8, :])
            nc.sync.dma_start(out=yt_sb[:, 4:8, :], in_=yt_r[:, 4:8, :])
            nc.gpsimd.dma_start(out=yv_sb[:, 8:16, :, :], in_=yv_d[:, 8:16, :, :])
            nc.gpsimd.dma_start(out=b1_sb, in_=b1_r)
            nc.gpsimd.dma_start(out=ws_sb, in_=ws_d)

            with (
                tc.tile_pool(name="psum_s", bufs=2, space="PSUM") as ps_pool,
                tc.tile_pool(name="psum_a", bufs=2, space="PSUM") as pa_pool,
                tc.tile_pool(name="work", bufs=3) as work,
                tc.tile_pool(name="pairs", bufs=2) as pairs,
                tc.tile_pool(name="vec", bufs=4) as vec,
                tc.tile_pool(name="stat", bufs=4) as stat,
            ):
                # =============== Phase A: attention ===============
                pend = None
                attn_ps = {}
                av2 = {}
                rc2 = {}
                pend_pair = []

                def pair_epilogue(ct, fast=False):
                    if fast:
                        # st slots free at phase A tail: K=1 PE broadcasts
                        # (one per head, dst partitions 0:D) avoid the DRAM
                        # roundtrip latency; the muls below are
                        # partition-shifted reads (v1-proven legal on DVE)
                        rbs = []
                        for hh in range(2):
                            rbh = ps_pool.tile(
                                [P, NT], F32, tag="st", name=f"rbps{hh}"
                            )
                            for nh in range(2):
                                sl = slice(nh * 512, (nh + 1) * 512)
                                nc.tensor.matmul(
                                    rbh[0:D, sl],
                                    ones_sb[0:1, 0:D],
                                    rc2[(ct, hh)][:, sl],
                                    start=True,
                                    stop=True,
                                )
                            rbs.append(rbh)
                    else:
                        nc.gpsimd.dma_start(
                            out=rc_dram[ct, 0:1, :], in_=rc2[(ct, 0)].bitcast(F32)
                        )
                        nc.gpsimd.dma_start(
                            out=rc_dram[ct, 1:2, :], in_=rc2[(ct, 1)].bitcast(F32)
                        )
                        rb = pairs.tile([P, NT], F32, tag="rb", name=f"rb{ct}")
                        nc.gpsimd.dma_start(
                            out=rb, in_=_pbcast2(rc_dram[ct, :, :], D)
                        )
                    dst = ht_sb[:, ct, :]
                    dstf = ht_sb[:, ct, :].bitcast(F32)
                    if fast:
                        nc.vector.tensor_mul(
                            ht_sb[0:D, ct, :], av2[ct][0:D, :], rbs[0][0:D, :]
                        )
                        nc.vector.tensor_mul(
                            ht_sb[D:P, ct, :], av2[ct][D:P, :], rbs[1][0:D, :]
                        )
                    else:
                        nc.vector.tensor_mul(dst, av2[ct], rb)
                    nc.gpsimd.tensor_add(
                        dst, dstf, xt_sb[:, ct, :].bitcast(F32)
                    )
                    nc.vector.tensor_mul(sq_sb[:, ct, :], dstf, dstf)
                    # yt chunk ct is dead now -> stream in the W1 chunk
                    nc.sync.dma_start(
                        out=w1_sb[:, ct, :], in_=w1_r[:, ct, :]
                    )

                for k in range(H * NMC + 1):
                    if k < H * NMC:
                        h, mc = divmod(k, NMC)
                        ct, off = h // 2, (h % 2) * D
                        st = ps_pool.tile([P, NT], F32, tag="st")
                        lhsT = yt_sb[off : off + D, ct, mc * P : (mc + 1) * P]
                        for nh in range(2):
                            sl = slice(nh * 512, (nh + 1) * 512)
                            nc.tensor.matmul(
                                st[:, sl],
                                lhsT,
                                xt_sb[off : off + D, ct, sl],
                                start=True,
                                stop=True,
                            )
                        e = work.tile([P, NT], BF16, tag="e")
                        nc.scalar.activation(e, st, AF.Exp, scale=SCALE)
                        cur = (h, mc, e)
                    else:
                        cur = None

                    if pend is not None:
                        h, mc, e = pend
                        ct, lo = h // 2, (h % 2) * D
                        if mc == 0:
                            attn_ps[h] = pa_pool.tile(
                                [D + 1, NT], F32, tag="at", name=f"at{h}"
                            )
                        ap_t = attn_ps[h]
                        lv = yv_sb[:, h, mc, :]
                        for nh in range(2):
                            sl = slice(nh * 512, (nh + 1) * 512)
                            nc.tensor.matmul(
                                ap_t[:, sl],
                                lv,
                                e[:, sl],
                                start=(mc == 0),
                                stop=(mc == NMC - 1),
                            )
                        if mc == NMC - 1:
                            if h % 2 == 0:
                                av2[ct] = pairs.tile(
                                    [P, NT], F32, tag="av", name=f"av2_{ct}"
                                )
                            rc2[(ct, h % 2)] = vec.tile(
                                [1, NT], F32R, tag="rc", name=f"rc_{h}"
                            )
                            nc.vector.tensor_copy(
                                av2[ct][lo : lo + D, :], ap_t[0:D, :]
                            )
                            with nc.allow_low_precision(reason="f32r denom"):
                                nc.vector.reciprocal(
                                    rc2[(ct, h % 2)], ap_t[D : D + 1, :]
                                )
                            del attn_ps[h]
                            if h % 2 == 1:
                                pend_pair.append((ct, k))
                    while pend_pair and (
                        cur is None or k - pend_pair[0][1] >= 4
                    ):
                        ctp, _ = pend_pair.pop(0)
                        pair_epilogue(ctp, fast=(ctp == NC - 1))
                    pend = cur

                # ====== LN stats (broadcast form) + short chain ======
                def ln_stats(src_sb, sqr_sb, idx, interleave=None):
                    """s1/s2 accumulation matmuls. If interleave is None runs
                    all chunks now; else caller drives per-chunk via the
                    returned closure. Returns (s1, s2) psum tiles."""
                    s1 = pa_pool.tile([P, NT], F32, tag="at", name=f"s1_{idx}")
                    s2 = pa_pool.tile([P, NT], F32, tag="at", name=f"s2_{idx}")

                    def chunk(ct, which=None):
                        if which in (None, 0):
                            for nh in range(2):
                                sl = slice(nh * 512, (nh + 1) * 512)
                                nc.tensor.matmul(
                                    s1[:, sl], ones_sb, src_sb[:, ct, sl],
                                    start=(ct == 0), stop=(ct == NC - 1),
                                )
                        if which in (None, 1):
                            for nh in range(2):
                                sl = slice(nh * 512, (nh + 1) * 512)
                                nc.tensor.matmul(
                                    s2[:, sl], ones_bf, sqr_sb[:, ct, sl],
                                    start=(ct == 0), stop=(ct == NC - 1),
                                )

                    if interleave is None:
                        for ct in range(NC):
                            chunk(ct)
                    return s1, s2, chunk

                def ln_chain(s1, s2, idx):
                    """Returns (nm_bc, rs_bc): [P,NT] broadcast -mean and
                    rstd. Frees s1/s2 psums early."""
                    c = 1.0 / DIM
                    nm_bc = stat.tile([P, NT], F32R, tag="st", name=f"nm_{idx}")
                    v_bc = stat.tile([P, NT], F32, tag="st", name=f"v_{idx}")
                    rs_bc = stat.tile([P, NT], F32R, tag="st", name=f"r_{idx}")
                    with nc.allow_low_precision(reason="f32r -mean"):
                        nc.scalar.activation(nm_bc, s1, AF.Copy, scale=-c)
                    nmf = nm_bc.bitcast(F32)
                    nc.vector.scalar_tensor_tensor(
                        v_bc, nmf, -1.0, nmf, OP.mult, OP.mult
                    )
                    nc.vector.scalar_tensor_tensor(
                        v_bc, s2, c, v_bc, OP.mult, OP.add
                    )
                    nc.scalar.activation(v_bc, v_bc, AF.Sqrt, bias=eps_t)
                    with nc.allow_low_precision(reason="f32r rstd"):
                        nc.vector.reciprocal(rs_bc, v_bc)
                    return nm_bc, rs_bc

                s1h, s2h, h_chunk = ln_stats(ht_sb, sq_sb, 0, interleave=True)
                for ct in range(NC - 1):
                    h_chunk(ct, which=0)
                for ct in range(NC - 1):
                    h_chunk(ct, which=1)

                # ====== Phase C: fused FFN on raw Hm + LN_o stats ======
                # O goes into xt_sb (over Hn; each Hn chunk's only reader is
                # its own residual add) so FFN's Hm rhs is never clobbered.
                # PE order: F(0), h_stats(7), F(1), rank1(0), F(2), rank1(1),
                # o_stats(0), ... so PE never waits on the DVE/ACT pipeline.
                s1o, s2o, o_chunk = ln_stats(xt_sb, sq_sb, 1, interleave=True)
                nm_h = rs_h = nm_row = None

                def hn(ct):
                    # most chunks go to Pool so the DVE queue stays clear for
                    # the u(oc) muls (which gate FFN psum recycling and relu)
                    dst = xt_sb[:, ct, :]
                    dstf = xt_sb[:, ct, :].bitcast(F32)
                    eng = nc.vector if ct in (2, 3) else nc.gpsimd
                    eng.tensor_add(
                        dst, ht_sb[:, ct, :].bitcast(F32), nm_h.bitcast(F32)
                    )
                    eng.tensor_mul(dst, dstf, rs_h.bitcast(F32))

                def rank1(oc):
                    for nh in range(2):
                        sl = slice(nh * 512, (nh + 1) * 512)
                        nc.tensor.matmul(
                            fps[oc][:, sl],
                            ws_sb[:, oc * P : (oc + 1) * P],
                            nm_row[:, sl],
                            start=False,
                            stop=True,
                        )

                def fcp(oc):
                    # ACT evacuates the FFN psum so its slot recycles at ACT
                    # pace instead of waiting on the DVE chain
                    t = work.tile([P, NT], F32, tag="f", name=f"fc{oc}")
                    nc.scalar.activation(t, fps[oc], AF.Copy)
                    del fps[oc]
                    fcps[oc] = t

                def finish_u(oc):
                    u = work.tile([P, NT], F32, tag="e", name=f"u{oc}")
                    nc.vector.tensor_mul(u, fcps.pop(oc), rs_h.bitcast(F32))
                    r = work.tile([P, NT], F32, tag="e", name=f"r{oc}")
                    nc.scalar.activation(
                        r, u, AF.Relu, bias=b1_sb[:, oc : oc + 1]
                    )
                    return r

                def finish_res(oc, r):
                    dst = xt_sb[:, oc, :]
                    dstf = xt_sb[:, oc, :].bitcast(F32)
                    eng = nc.gpsimd if oc in (1, 4, 6) else nc.vector
                    eng.tensor_add(dst, dstf, r)
                    eng2 = nc.vector if oc in (1, 4, 6) else nc.gpsimd
                    eng2.tensor_mul(sq_sb[:, oc, :], dstf, dstf)
                    nc.sync.dma_start(out=ot_r[:, oc, :], in_=xt_sb[:, oc, :])

                fps = {}
                fcps = {}
                for oc in range(NC):
                    fps[oc] = ps_pool.tile([P, NT], F32, tag="st", name=f"fps{oc}")
                    for kc in range(NC):
                        lhsT = w1_sb[:, kc, oc * P : (oc + 1) * P]
                        for nh in range(2):
                            sl = slice(nh * 512, (nh + 1) * 512)
                            nc.tensor.matmul(
                                fps[oc][:, sl],
                                lhsT,
                                ht_sb[:, kc, sl],
                                start=(kc == 0),
                                stop=False,
                            )
                    if oc == 0:
                        c = 1.0 / DIM
                        h_chunk(NC - 1, which=0)
                        # chain part A: only needs the s1 stop
                        nm_h = stat.tile([P, NT], F32R, tag="st", name="nm_h")
                        v_bc = stat.tile([P, NT], F32, tag="st", name="v_h")
                        rs_h = stat.tile([P, NT], F32R, tag="st", name="rs_h")
                        with nc.allow_low_precision(reason="f32r -mean"):
                            nc.scalar.activation(nm_h, s1h, AF.Copy, scale=-c)
                        nmf = nm_h.bitcast(F32)
                        nc.vector.scalar_tensor_tensor(
                            v_bc, nmf, -1.0, nmf, OP.mult, OP.mult
                        )
                        nm_row = nm_h[0:1, :]
                        rank1(0)
                        fcp(0)
                        h_chunk(NC - 1, which=1)
                        # chain part B: needs the s2 stop
                        nc.vector.scalar_tensor_tensor(
                            v_bc, s2h, c, v_bc, OP.mult, OP.add
                        )
                        nc.scalar.activation(v_bc, v_bc, AF.Sqrt, bias=eps_t)
                        with nc.allow_low_precision(reason="f32r rstd"):
                            nc.vector.reciprocal(rs_h, v_bc)
                        hn(0)
                    else:
                        rank1(oc)
                        fcp(oc)
                        rr = finish_u(oc - 1)
                        hn(oc)
                        finish_res(oc - 1, rr)
                        if oc >= 2:
                            o_chunk(oc - 2)
                rr = finish_u(NC - 1)
                finish_res(NC - 1, rr)
                o_chunk(NC - 2)
                o_chunk(NC - 1)

                # ====== Phase D: ship raw stat rows; host applies LN_o ====
                r1 = vec.tile([1, NT], F32, tag="rc", name="r1")
                r2 = vec.tile([1, NT], F32, tag="rc", name="r2")
                nc.vector.tensor_copy(r1, s1o[0:1, :])
                nc.scalar.activation(r2, s2o[0:1, :], AF.Copy)
                nc.sync.dma_start(out=st1_d, in_=r1)
                nc.sync.dma_start(out=st2_d, in_=r2)

        for free in reversed(_frees):
            free()

    nc.finalize()
    return nc


@functools.lru_cache(maxsize=4)
def _program(n_cores: int, reps: int = 1):
    return build_program(n_cores, reps)


def _prep_core(Xb, Yb):
    import ml_dtypes

    xt = np.ascontiguousarray(Xb.T)
    yt = np.ascontiguousarray(Yb.T)
    # [P, H, NMC, D+1]: partition-major, matching the SBUF tile exactly
    yv = np.empty((P, H, NMC, D + 1), np.float32)
    v = Yb.reshape(NMC, P, H, D)  # m = mc*128 + p
    yv[:, :, :, :D] = v.transpose(1, 2, 0, 3)
    yv[:, :, :, D] = 1.0
    return xt, yt, yv.astype(ml_dtypes.bfloat16)


def kernel(X, Y, W1, b1, gamma_h, beta_h, gamma_o, beta_o, num_heads):
    X = np.asarray(X, np.float32)
    Y = np.asarray(Y, np.float32)
    W1 = np.asarray(W1, np.float32)
    b1 = np.asarray(b1, np.float32)
    gamma_h = np.asarray(gamma_h, np.float32)
    beta_h = np.asarray(beta_h, np.float32)
    gamma_o = np.asarray(gamma_o, np.float32)
    beta_o = np.asarray(beta_o, np.float32)
    B, n, dim = X.shape
    assert (B, n, dim) == (8, NT, DIM) and int(num_heads) == H

    affine_h = bool(not (np.all(gamma_h == 1.0) and np.all(beta_h == 0.0)))
    affine_o = bool(not (np.all(gamma_o == 1.0) and np.all(beta_o == 0.0)))
    assert not affine_h, "v3 kernel only supports non-affine LN_h"

    ws = W1.sum(axis=0, keepdims=True)
    nc = _program(B)
    in_maps = []
    for b in range(B):
        xt, yt, yv = _prep_core(X[b], Y[b])
        in_maps.append(
            {"xt": xt, "yt": yt, "yv": yv, "w1": W1, "b1": b1, "ws": ws}
        )

    res = run_bass_kernel_spmd(nc, in_maps, list(range(B)))

    out = np.empty((B, NT, DIM), np.float32)
    for b in range(B):
        s1 = res.results[b]["st1"][0].astype(np.float64)
        s2 = res.results[b]["st2"][0].astype(np.float64)
        mean = s1 / DIM
        var = s2 / DIM - mean * mean
        rs = (1.0 / np.sqrt(var + EPS)).astype(np.float32)[:, None]
        bv = (-mean).astype(np.float32)[:, None] * rs
        o = res.results[b]["ot"].T * rs + bv
        if affine_o:
            o = o * gamma_o[None, :] + beta_o[None, :]
        out[b] = o
    return out
